# revision 2
# baseline (speedup 1.0000x reference)
"""CenterLoss Trainium2 kernel.

loss = mean_b clip(||x_b - centers[labels_b]||^2, 1e-12, 1e12)

Shapes (hardcoded): x [8192, 512] f32, labels [8192] int64 in [0, 10000),
centers [10000, 512] f32.  Output: f32 scalar.

Strategy: data-parallel over batch across 8 cores (1024 rows each);
centers stay in HBM (replicated input) and each core gathers exactly the
1024 rows it needs.  Only the diagonal entries distmat[b, labels_b] of the
reference's [B, C] distmat are needed, so the kernel is memory-bound.

v2 changes vs the indirect-DMA baseline (29.4 us):
  - x and centers are cast to bf16 on the host: halves HBM traffic
    (2 MB/core instead of 4 MB) and doubles DVE throughput.  The rel-err
    budget (2e-2) dwarfs the ~1e-4 this costs.
  - The 8 serialized `indirect_dma_start`s (one 12 us Q7 DRAIN between
    them) are replaced by 4 `dma_gather` chunks of 256 rows each - the
    SWDGE gather ucode path.  Labels are pre-permuted on the host so the
    gather's fixed dst layout (row j -> partition j%128, block j//128)
    coincides with the kernel's (row p*T+t -> partition p, block t).
  - x is loaded in 4 matching chunks so DVE/ACT compute on chunk k
    overlaps the DMA of chunk k+1.

Per tile [128, 512]: DVE subtract, then ACT Square with row-accumulate
into dist[p, t].  The [128, 8] per-row distances go back to the host,
which applies clip and the global mean.
"""

import sys

import numpy as np

try:
    import concourse  # noqa: F401
except ImportError:  # pragma: no cover
    sys.path.insert(0, "/opt/trn_rl_repo")

import ml_dtypes

B, D, C = 8192, 512, 10000
N_CORES = 8
P = 128
ROWS = B // N_CORES  # 1024 rows per core
T = ROWS // P        # 8 tiles of 128 rows
NCHUNK = 4           # gather/x-load chunks (2 tiles each)
TPC = T // NCHUNK    # tiles per chunk

CLAMP_MIN = 1e-12
CLAMP_MAX = 1e12

_CACHE = {}


def _build():
    import concourse.bacc as bacc
    import concourse.tile as tile
    from concourse import bass, mybir

    f32 = mybir.dt.float32
    bf16 = mybir.dt.bfloat16
    i16 = mybir.dt.int16

    nc = bacc.Bacc("TRN2", target_bir_lowering=False, num_devices=N_CORES)
    x = nc.dram_tensor("x", [ROWS, D], bf16, kind="ExternalInput")
    labels16 = nc.dram_tensor("labels16", [P, ROWS // 16], i16, kind="ExternalInput")
    centers = nc.dram_tensor("centers", [C, D], bf16, kind="ExternalInput")
    out = nc.dram_tensor("out", [P, T], f32, kind="ExternalOutput")

    with tile.TileContext(nc) as tc:
        with (
            tc.tile_pool(name="big", bufs=1) as big,
            tc.tile_pool(name="small", bufs=1) as small,
            tc.tile_pool(name="work", bufs=4) as work,
        ):
            idx = small.tile([P, ROWS // 16], i16)
            dist = small.tile([P, T], f32)
            xbig = big.tile([P, T * D], bf16)
            cbig = big.tile([P, T * D], bf16)

            # idx wraps gather order j -> [j%16, j//16], replicated x8 down
            # the partitions (prepared host-side).  128 B per partition.
            nc.sync.dma_start(out=idx[:], in_=labels16[:, :])

            # xbig[p, t*D:(t+1)*D] = x[p*T + t, :]; chunk k covers
            # t in [k*TPC, (k+1)*TPC) = TPC*1 KB contiguous per partition.
            xsrc = x[:, :].rearrange("(p t) d -> p (t d)", p=P)
            for k in range(NCHUNK):
                sl = slice(k * TPC * D, (k + 1) * TPC * D)
                nc.sync.dma_start(out=xbig[:, sl], in_=xsrc[:, sl])

            # cbig chunk k = centers[idx[j]] for j in [k*256, (k+1)*256):
            # lands at partition j%128, tile block j//128 - matching xbig
            # because the host permuted the labels.
            for k in range(NCHUNK):
                nidx = TPC * P  # 256 rows per gather
                cdst = cbig[:, k * TPC * D : (k + 1) * TPC * D].rearrange(
                    "p (t d) -> p t d", d=D
                )
                nc.gpsimd.dma_gather(
                    cdst,
                    centers[:, :],
                    idx[:, k * (nidx // 16) : (k + 1) * (nidx // 16)],
                    nidx,
                    nidx,
                    D,
                )

            for t in range(T):
                sl = slice(t * D, (t + 1) * D)
                diff = work.tile([P, D], bf16, tag="diff")
                sq = work.tile([P, D], bf16, tag="sq")
                nc.vector.tensor_sub(diff[:], xbig[:, sl], cbig[:, sl])
                # sq = diff^2 on ACT; dist[:, t] = row-sum(sq) via ACT accum
                # (f32).  ACT and DVE pipeline across tiles.
                nc.scalar.activation(
                    sq[:],
                    diff[:],
                    mybir.ActivationFunctionType.Square,
                    accum_out=dist[:, t : t + 1],
                )
            nc.sync.dma_start(out=out[:, :], in_=dist[:])

    nc.compile()
    return nc


def get_nc():
    nc = _CACHE.get("nc")
    if nc is None:
        nc = _CACHE["nc"] = _build()
    return nc


def make_labels16(labels_shard):
    """[1024] int labels -> [128, 64] int16 gather-index buffer.

    Gather order j = t*128 + p must fetch labels_shard[p*T + t] so the
    gather dst (row j -> partition j%128, block j//128) matches xbig's
    layout (x row p*T+t -> partition p, block t).  The wrap puts idx j at
    [j%16, j//16], replicated to all 8 16-partition groups (one per Q7
    core).
    """
    idx = labels_shard.astype(np.int16).reshape(P, T).T.reshape(-1)  # j = t*P+p
    buf = idx.reshape(ROWS // 16, 16).T  # [16, 64], buf[q, s] = idx[s*16+q]
    return np.ascontiguousarray(np.tile(buf, (8, 1)))  # [128, 64]


def make_in_maps(x, labels, centers):
    x16 = np.asarray(x).astype(ml_dtypes.bfloat16)
    c16 = np.ascontiguousarray(np.asarray(centers).astype(ml_dtypes.bfloat16))
    labels = np.asarray(labels)
    in_maps = []
    for i in range(N_CORES):
        lo, hi = i * ROWS, (i + 1) * ROWS
        in_maps.append(
            {
                "x": np.ascontiguousarray(x16[lo:hi]),
                "labels16": make_labels16(labels[lo:hi]),
                "centers": c16,
            }
        )
    return in_maps


def finish(per_core_outs):
    """per_core_outs: list of 8 [P, T] arrays -> f32 scalar loss."""
    d = np.concatenate([np.asarray(o).reshape(-1) for o in per_core_outs])
    d = np.clip(d, CLAMP_MIN, CLAMP_MAX)
    return np.asarray(np.mean(d, dtype=np.float64), dtype=np.float32)


def kernel(x, labels, centers):
    from concourse.bass_utils import run_bass_kernel_spmd

    nc = get_nc()
    in_maps = make_in_maps(x, labels, centers)
    res = run_bass_kernel_spmd(nc, in_maps, core_ids=list(range(N_CORES)))
    return finish([r["out"] for r in res.results])


# revision 3
# speedup vs baseline: 1.3497x; 1.3497x over previous
"""CenterLoss Trainium2 kernel.

loss = mean_b clip(||x_b - centers[labels_b]||^2, 1e-12, 1e12)

Shapes (hardcoded): x [8192, 512] f32, labels [8192] int64 in [0, 10000),
centers [10000, 512] f32.  Output: f32 scalar.

Strategy: data-parallel over batch across 8 cores (1024 rows each);
centers stay in HBM (replicated input) and each core gathers exactly the
1024 rows it needs with indirect DMAs (labels as row offsets).  Only the
diagonal entries distmat[b, labels_b] of the reference's [B, C] distmat
are needed, so the kernel is memory/Q7-bound, never compute-bound.

v3 changes vs the single-queue indirect-DMA baseline (29.4 us):
  - x and centers are cast to bf16 on the host: halves HBM traffic and
    doubles DVE throughput.  Costs ~1e-5 rel err against a 2e-2 budget.
  - The 8 indirect gathers (128 rows each) are spread across 4 SWDGE
    queue contexts (qPoolDynamic..qPoolDynamic3) instead of serializing
    ~1.1 us each on one queue's Q7 descriptor-generation path.
  - x is loaded in 4 chunks split across both HWDGE rings (sync + ACT)
    so compute on tile t can start as soon as chunk t//2 and gather t
    have landed.

Per-core layout: batch row r = p*8 + t maps to SBUF partition p, tile t
(8 tiles of [128, 512]).  Per tile: DVE subtract then ACT Square with
row-accumulate into dist[p, t].  The [128, 8] per-row distances go back
to the host, which applies clip and the global mean.
"""

import sys

import numpy as np

try:
    import concourse  # noqa: F401
except ImportError:  # pragma: no cover
    sys.path.insert(0, "/opt/trn_rl_repo")

import ml_dtypes

B, D, C = 8192, 512, 10000
N_CORES = 8
P = 128
ROWS = B // N_CORES  # 1024 rows per core
T = ROWS // P        # 8 tiles of 128 rows
NQ = 4               # SWDGE queues used for the gathers

CLAMP_MIN = 1e-12
CLAMP_MAX = 1e12

_CACHE = {}


def _indirect_dma_on_queue(gp, out, in_, offset_ap, queue: str):
    """nc.gpsimd.indirect_dma_start (src-indirect, axis 0) with a queue
    override - bass hardcodes queue="qPoolDynamic", serializing all
    indirect DMAs on one SWDGE context."""
    from concourse import bass, mybir

    assert in_.space == bass.MemorySpace.DRAM
    assert out.space == bass.MemorySpace.SBUF
    assert isinstance(in_.offset, int) and in_.offset == 0

    out_ap = gp.lower_ap_dma(out, for_indirect_dma=True)
    in_ap = gp.lower_ap_dma(in_, for_indirect_dma=True)
    assert len(in_ap) == 1 and len(out_ap) == 1

    lowered_offset = gp.lower_ap_dma(offset_ap)
    assert len(lowered_offset) == 1
    in_ap.append(lowered_offset[0])

    ap_shape = in_.shape
    coef = 1
    for i in range(1, len(ap_shape)):
        coef *= ap_shape[i]

    in_ap[0].dynamic_ap_info = mybir.DynamicAccessPatternInfo(
        c=0,
        actual_ap=out.ap,
        indirect_dim_max_index=ap_shape[0],
        offset_expr=[
            mybir.DynamicAccessPatternOffsetExpr(
                coef=coef,
                aff_expr=mybir.DynamicAccessPatternOffsetExprAffExpr(
                    kind="IndirectArgId", arg_id=1
                ),
            )
        ],
    )

    return gp.add_instruction(
        mybir.InstDMACopy(
            name=gp.bass.get_next_instruction_name(),
            queue=queue,
            mode="Copy",
            ins=in_ap,
            outs=out_ap,
            oob_is_err=True,
            cce_op=mybir.AluOpType.bypass,
        )
    )


def _build():
    import concourse.bacc as bacc
    import concourse.tile as tile
    from concourse import bass, mybir

    f32 = mybir.dt.float32
    bf16 = mybir.dt.bfloat16
    i32 = mybir.dt.int32

    nc = bacc.Bacc(
        "TRN2", target_bir_lowering=False, num_devices=N_CORES, num_swdge_queues=NQ
    )
    x = nc.dram_tensor("x", [ROWS, D], bf16, kind="ExternalInput")
    labels = nc.dram_tensor("labels", [ROWS, 1], i32, kind="ExternalInput")
    centers = nc.dram_tensor("centers", [C, D], bf16, kind="ExternalInput")
    out = nc.dram_tensor("out", [P, T], f32, kind="ExternalOutput")

    queue_names = ["qPoolDynamic"] + [f"qPoolDynamic{i}" for i in range(1, NQ)]

    with tile.TileContext(nc) as tc:
        with (
            tc.tile_pool(name="big", bufs=1) as big,
            tc.tile_pool(name="small", bufs=1) as small,
            tc.tile_pool(name="work", bufs=4) as work,
        ):
            idx = small.tile([P, T], i32)
            dist = small.tile([P, T], f32)
            xbig = big.tile([P, T * D], bf16)
            cbig = big.tile([P, T * D], bf16)

            # idx[p, t] = labels[p*T + t]; 32 B contiguous per partition.
            nc.sync.dma_start(
                out=idx[:], in_=labels[:, :].rearrange("(p t) o -> p (t o)", p=P)
            )
            # xbig[p, t*D:(t+1)*D] = x[p*T + t, :]; 4 chunks of 2 tiles,
            # alternating between the two HWDGE rings (sync=SP, scalar=ACT).
            xsrc = x[:, :].rearrange("(p t) d -> p (t d)", p=P)
            for k in range(4):
                sl = slice(k * 2 * D, (k + 1) * 2 * D)
                eng = nc.sync if k % 2 == 0 else nc.scalar
                eng.dma_start(out=xbig[:, sl], in_=xsrc[:, sl])

            # cbig[p, t*D:(t+1)*D] = centers[idx[p, t], :].  One indirect
            # DMA per 128 rows, spread across the 4 SWDGE queues.
            for t in range(T):
                sl = slice(t * D, (t + 1) * D)
                _indirect_dma_on_queue(
                    nc.gpsimd,
                    cbig[:, sl],
                    centers[:, :],
                    idx[:, t : t + 1],
                    queue_names[t % NQ],
                )
            for t in range(T):
                sl = slice(t * D, (t + 1) * D)
                diff = work.tile([P, D], bf16, tag="diff")
                sq = work.tile([P, D], bf16, tag="sq")
                nc.vector.tensor_sub(diff[:], xbig[:, sl], cbig[:, sl])
                # sq = diff^2 on ACT; dist[:, t] = row-sum(sq) via ACT accum.
                nc.scalar.activation(
                    sq[:],
                    diff[:],
                    mybir.ActivationFunctionType.Square,
                    accum_out=dist[:, t : t + 1],
                )
            nc.sync.dma_start(out=out[:, :], in_=dist[:])

    nc.compile()
    return nc


def get_nc():
    nc = _CACHE.get("nc")
    if nc is None:
        nc = _CACHE["nc"] = _build()
    return nc


def make_in_maps(x, labels, centers):
    labels_i32 = np.ascontiguousarray(
        np.asarray(labels).astype(np.int32).reshape(B, 1)
    )
    x16 = np.asarray(x).astype(ml_dtypes.bfloat16)
    c16 = np.ascontiguousarray(np.asarray(centers).astype(ml_dtypes.bfloat16))
    in_maps = []
    for i in range(N_CORES):
        lo, hi = i * ROWS, (i + 1) * ROWS
        in_maps.append(
            {
                "x": np.ascontiguousarray(x16[lo:hi]),
                "labels": labels_i32[lo:hi],
                "centers": c16,
            }
        )
    return in_maps


def finish(per_core_outs):
    """per_core_outs: list of 8 [P, T] arrays -> f32 scalar loss."""
    d = np.concatenate([np.asarray(o).reshape(-1) for o in per_core_outs])
    d = np.clip(d, CLAMP_MIN, CLAMP_MAX)
    return np.asarray(np.mean(d, dtype=np.float64), dtype=np.float32)


def kernel(x, labels, centers):
    from concourse.bass_utils import run_bass_kernel_spmd

    nc = get_nc()
    in_maps = make_in_maps(x, labels, centers)
    res = run_bass_kernel_spmd(nc, in_maps, core_ids=list(range(N_CORES)))
    return finish([r["out"] for r in res.results])


# revision 4
# speedup vs baseline: 1.5737x; 1.1660x over previous
"""CenterLoss Trainium2 kernel.

loss = mean_b clip(||x_b - centers[labels_b]||^2, 1e-12, 1e12)

Shapes (hardcoded): x [8192, 512] f32, labels [8192] int64 in [0, 10000),
centers [10000, 512] f32.  Output: f32 scalar.

Strategy: data-parallel over batch across 8 cores (1024 rows each);
centers stay in HBM (replicated input) and each core gathers exactly the
1024 rows it needs with indirect DMAs (labels as row offsets).  Only the
diagonal entries distmat[b, labels_b] of the reference's [B, C] distmat
are needed, so the kernel is memory-bound.

v4 changes vs the 29.4 us baseline:
  - x and centers are cast to bf16 on the host: halves HBM traffic and
    doubles DVE throughput (~1e-5 rel err against a 2e-2 budget).
  - The gather is 4 indirect DMAs of 256 rows (offset AP [128, 2])
    instead of 8 of 128 rows.  SWDGE cost is ~1 us fixed per
    *instruction* + 0.34 ns/descriptor, so halving the instruction count
    halves the serial Q7 descriptor-generation chain that dominated the
    baseline.
  - Square+reduce is split across engines: tiles 0-5 on ACT (Square
    activation with row-accumulate), tiles 6-7 on DVE (mult + reduce),
    balancing the two pipelines (~5 us each) instead of serializing
    everything behind ACT (7.2 us).

Per-core layout: batch row r = p*8 + t maps to SBUF partition p, tile t
(8 tiles of [128, 512]).  The [128, 8] per-row distances go back to the
host, which applies clip and the global mean.
"""

import sys

import numpy as np

try:
    import concourse  # noqa: F401
except ImportError:  # pragma: no cover
    sys.path.insert(0, "/opt/trn_rl_repo")

import ml_dtypes

B, D, C = 8192, 512, 10000
N_CORES = 8
P = 128
ROWS = B // N_CORES  # 1024 rows per core
T = ROWS // P        # 8 tiles of 128 rows
TPG = 2              # tiles per gather instruction
ACT_TILES = 6        # tiles 0..5 reduce on ACT, the rest on DVE

CLAMP_MIN = 1e-12
CLAMP_MAX = 1e12

_CACHE = {}


def _build():
    import concourse.bacc as bacc
    import concourse.tile as tile
    from concourse import bass, mybir

    f32 = mybir.dt.float32
    bf16 = mybir.dt.bfloat16
    i32 = mybir.dt.int32

    nc = bacc.Bacc("TRN2", target_bir_lowering=False, num_devices=N_CORES)
    x = nc.dram_tensor("x", [ROWS, D], bf16, kind="ExternalInput")
    labels = nc.dram_tensor("labels", [ROWS, 1], i32, kind="ExternalInput")
    centers = nc.dram_tensor("centers", [C, D], bf16, kind="ExternalInput")
    out = nc.dram_tensor("out", [P, T], f32, kind="ExternalOutput")

    with tile.TileContext(nc) as tc:
        with (
            tc.tile_pool(name="big", bufs=1) as big,
            tc.tile_pool(name="small", bufs=1) as small,
            tc.tile_pool(name="work", bufs=4) as work,
        ):
            idx = small.tile([P, T], i32)
            dist = small.tile([P, T], f32)
            xbig = big.tile([P, T * D], bf16)
            cbig = big.tile([P, T * D], bf16)

            # idx[p, t] = labels[p*T + t]; 32 B contiguous per partition.
            nc.sync.dma_start(
                out=idx[:], in_=labels[:, :].rearrange("(p t) o -> p (t o)", p=P)
            )
            # xbig[p, t*D:(t+1)*D] = x[p*T + t, :]; 4 chunks of 2 tiles,
            # alternating between the two HWDGE rings (sync=SP, scalar=ACT).
            xsrc = x[:, :].rearrange("(p t) d -> p (t d)", p=P)
            for k in range(4):
                sl = slice(k * 2 * D, (k + 1) * 2 * D)
                eng = nc.sync if k % 2 == 0 else nc.scalar
                eng.dma_start(out=xbig[:, sl], in_=xsrc[:, sl])

            # cbig[p, t*D:(t+1)*D] = centers[idx[p, t], :].  One indirect
            # DMA per TPG tiles: offset AP [128, TPG] ravels in the same
            # (p, t) order as the [128, TPG*D] destination.
            for g in range(T // TPG):
                tsl = slice(g * TPG, (g + 1) * TPG)
                dsl = slice(g * TPG * D, (g + 1) * TPG * D)
                nc.gpsimd.indirect_dma_start(
                    out=cbig[:, dsl],
                    out_offset=None,
                    in_=centers[:, :],
                    in_offset=bass.IndirectOffsetOnAxis(ap=idx[:, tsl], axis=0),
                )
            for t in range(T):
                sl = slice(t * D, (t + 1) * D)
                diff = work.tile([P, D], bf16, tag="diff")
                nc.vector.tensor_sub(diff[:], xbig[:, sl], cbig[:, sl])
                if t < ACT_TILES:
                    # sq = diff^2 on ACT; dist[:, t] = row-sum via ACT accum.
                    sq = work.tile([P, D], bf16, tag="sq")
                    nc.scalar.activation(
                        sq[:],
                        diff[:],
                        mybir.ActivationFunctionType.Square,
                        accum_out=dist[:, t : t + 1],
                    )
                else:
                    # DVE path: mult + row-reduce, balancing the ACT queue.
                    sq = work.tile([P, D], bf16, tag="sq")
                    nc.vector.tensor_tensor(
                        out=sq[:], in0=diff[:], in1=diff[:], op=mybir.AluOpType.mult
                    )
                    nc.vector.tensor_reduce(
                        out=dist[:, t : t + 1],
                        in_=sq[:],
                        axis=mybir.AxisListType.X,
                        op=mybir.AluOpType.add,
                    )
            nc.sync.dma_start(out=out[:, :], in_=dist[:])

    nc.compile()
    return nc


def get_nc():
    nc = _CACHE.get("nc")
    if nc is None:
        nc = _CACHE["nc"] = _build()
    return nc


def make_in_maps(x, labels, centers):
    labels_i32 = np.ascontiguousarray(
        np.asarray(labels).astype(np.int32).reshape(B, 1)
    )
    x16 = np.asarray(x).astype(ml_dtypes.bfloat16)
    c16 = np.ascontiguousarray(np.asarray(centers).astype(ml_dtypes.bfloat16))
    in_maps = []
    for i in range(N_CORES):
        lo, hi = i * ROWS, (i + 1) * ROWS
        in_maps.append(
            {
                "x": np.ascontiguousarray(x16[lo:hi]),
                "labels": labels_i32[lo:hi],
                "centers": c16,
            }
        )
    return in_maps


def finish(per_core_outs):
    """per_core_outs: list of 8 [P, T] arrays -> f32 scalar loss."""
    d = np.concatenate([np.asarray(o).reshape(-1) for o in per_core_outs])
    d = np.clip(d, CLAMP_MIN, CLAMP_MAX)
    return np.asarray(np.mean(d, dtype=np.float64), dtype=np.float32)


def kernel(x, labels, centers):
    from concourse.bass_utils import run_bass_kernel_spmd

    nc = get_nc()
    in_maps = make_in_maps(x, labels, centers)
    res = run_bass_kernel_spmd(nc, in_maps, core_ids=list(range(N_CORES)))
    return finish([r["out"] for r in res.results])
